# revision 44
# baseline (speedup 1.0000x reference)
"""Trainium2 Bass kernel for nn_AFF_MambaLayer (bi-directional selective scan).

Sharding: one depth-slice (1024 tokens) per core (8 cores). Each core runs
both scan directions for all 192 channels over its slice. Cross-slice decay
is numerically dead (exp(-~0.55*1024)), so the full-sequence scan equals the
local scan plus a WIN-token boundary correction driven by the *neighbor*
core's final state only; finals are exchanged with two small AllGathers.

Partition convention: the 192-channel dimension is split 128 + 64, with the
64-row half stored in [128, *] tiles at base partition 64 (hardware compute
ops require all operands to share a start partition in {0, 32, 64}).
"""
import os
import sys

import numpy as np

sys.path.insert(0, "/opt/trn_rl_repo")

# geometry
C = 96
DIN = 192
N = 16
R = 6
NS = 8           # slices == cores
SL = 1024        # tokens per slice
TP = SL + 3      # with conv left-context
NT = 24          # scan tiles per direction (8 d-channels x 16 states each)
WIN = 128        # correction window (carry decay is dead past ~64)

_cache = {}

# ---- const blob layouts (col offsets), shared host/device ----
_F32_ITEMS = [
    ("padfix_a", 3), ("padfix_b", 3), ("convw_a", 4), ("convw_b", 4),
    ("bias_u_a", 1), ("bias_u_b", 1), ("bias_z_a", 1), ("bias_z_b", 1),
    ("dtb_f_a", 1), ("dtb_f_b", 1), ("dtb_r_a", 1), ("dtb_r_b", 1),
    ("Arep_f", NT), ("Arep_r", NT),
    ("sel_f", 1), ("sel_r", 1),
    ("ident", 128), ("Dsum_a", 1), ("Dsum_b", 1), ("fusb", 1),
]
_F32_OFF = {}
_off = 0
for _nm, _nc in _F32_ITEMS:
    _F32_OFF[_nm] = (_off, _nc)
    _off += _nc
F32_COLS = _off

# f32r blob: constants consumed as matmul stationaries
_F32R_ITEMS = [
    ("oh16s", 128),
    ("xprojT_f_a", 70), ("xprojT_f_b", 70),
    ("xprojT_r_a", 70), ("xprojT_r_b", 70),
    ("dtwT_f", 256), ("dtwT_r", 256),
] + [(f"ohs{v}", 128) for v in range(8)]
_F32R_OFF = {}
_off = 0
for _nm, _nc in _F32R_ITEMS:
    _F32R_OFF[_nm] = (_off, _nc)
    _off += _nc
F32R_COLS = _off

_BF_ITEMS = [("outpT_a", C), ("outpT_b", C), ("fuswT", C)] + [
    (f"red128b_{t}", 128) for t in range(16)] + [
    (f"ohsb{v}", 128) for v in range(8)]
_BF_OFF = {}
_off = 0
for _nm, _nc in _BF_ITEMS:
    _BF_OFF[_nm] = (_off, _nc)
    _off += _nc
BF_COLS = _off


def _build_graph(dbg=False):
    import concourse.bass as bass
    import concourse.bacc as bacc
    import concourse.mybir as mybir
    from concourse import tile

    FP32 = mybir.dt.float32
    F32R = mybir.dt.float32r
    BF16 = mybir.dt.bfloat16
    AF = mybir.ActivationFunctionType
    OP = mybir.AluOpType

    nc = bacc.Bacc("TRN2", target_bir_lowering=False, debug=False, num_devices=NS)

    P = {}

    def inp(name, shape, dt=FP32):
        P[name] = nc.dram_tensor(name, list(shape), dt, kind="ExternalInput").ap()

    inp("x_sl", [C, TP], F32R)
    inp("mean96", [C, C], F32R)
    inp("w1T", [C, 512], F32R)
    inp("blobf", [128, F32_COLS])
    inp("blobr", [128, F32R_COLS], F32R)
    inp("blobb", [128, BF_COLS], BF16)

    out_t = nc.dram_tensor("out", [C, SL], FP32, kind="ExternalOutput").ap()
    dbg_t = {}
    if dbg:
        for name, shape in [
            ("dbg_u", [DIN, SL]), ("dbg_g", [DIN, SL]), ("dbg_dt_f", [DIN, SL]),
            ("dbg_h0_f", [128, SL]), ("dbg_yslg", [DIN, SL]),
            ("dbg_hin_f", [128, NT]), ("dbg_hin_r", [128, NT]),
            ("dbg_yc_f", [128, 2 * WIN]), ("dbg_osl", [C, SL]),
        ]:
            dbg_t[name] = nc.dram_tensor(name, shape, FP32, kind="ExternalOutput").ap()

    RG = [list(range(NS))]

    with tile.TileContext(nc) as tc:
        with (
            tc.tile_pool(name="const", bufs=1) as cst,
            tc.tile_pool(name="pers", bufs=1) as pers,
            tc.tile_pool(name="wk", bufs=3) as wk,
            tc.tile_pool(name="psw", bufs=3, space="PSUM") as psw,
            tc.tile_pool(name="psc", bufs=1, space="PSUM") as psc,
            tc.tile_pool(name="dram", bufs=1, space="DRAM") as drp,
        ):
            # x slice first so LayerNorm can start ASAP
            x_sbr = pers.tile([C, TP], F32R, name="x_sb", tag="x_sb")
            nc.sync.dma_start(x_sbr[:, :], P["x_sl"])
            x_sb = x_sbr.bitcast(FP32)
            mean96 = cst.tile([C, C], F32R, name="mean96", tag="mean96")
            nc.sync.dma_start(mean96[:, :], P["mean96"])
            w1T = cst.tile([C, 512], F32R, name="w1T", tag="w1T")
            nc.sync.dma_start(w1T[:, :], P["w1T"])
            blobf = cst.tile([128, F32_COLS], FP32, name="blobf", tag="blobf")
            nc.sync.dma_start(blobf[:, :], P["blobf"])
            blobr = cst.tile([128, F32R_COLS], F32R, name="blobr", tag="blobr")
            nc.sync.dma_start(blobr[:, :], P["blobr"])
            blobb = cst.tile([128, BF_COLS], BF16, name="blobb", tag="blobb")
            nc.sync.dma_start(blobb[:, :], P["blobb"])

            def cf(nm, rows=None):
                o, ncol = _F32_OFF[nm]
                t = blobf[:, o:o + ncol] if rows is None else \
                    blobf[rows[0]:rows[1], o:o + ncol]
                return t

            def cfr(nm, rows=None):
                o, ncol = _F32R_OFF[nm]
                return blobr[:, o:o + ncol] if rows is None else \
                    blobr[rows[0]:rows[1], o:o + ncol]

            def cb(nm, rows=None):
                o, ncol = _BF_OFF[nm]
                return blobb[:, o:o + ncol] if rows is None else \
                    blobb[rows[0]:rows[1], o:o + ncol]

            def v64(pool, name, cols, tag, bufs=None, dt=FP32):
                """[64, cols] logical tile stored at base partition 64."""
                kw = dict(name=name, tag=tag)
                if bufs is not None:
                    kw["bufs"] = bufs
                t = pool.tile([128, cols], dt, **kw)
                return t[64:128]

            # persistent activations (second halves at base 64)
            g0 = pers.tile([128, SL], BF16, name="g0", tag="g0")
            g1 = v64(pers, "g1", SL, "g1", dt=BF16)
            u0 = pers.tile([128, SL], F32R, name="u0", tag="u0")
            u1 = v64(pers, "u1", SL, "u1", dt=F32R)
            u0f = u0.bitcast(FP32)
            u1f = u1.bitcast(FP32)

            # ---------------- preprocessing (scoped pool) ----------------
            with tc.tile_pool(name="pre", bufs=1) as pre:
                xsq = pre.tile([C, TP], F32R, name="xsq", tag="xsq")
                nc.scalar.square(xsq[:, :], x_sb[:, :])
                x_r = x_sbr
                mu_b = pre.tile([C, TP], FP32, name="mu_b", tag="mu_b")
                msq_b = pre.tile([C, TP], FP32, name="msq_b", tag="msq_b")
                for c0, cn in ((0, 512), (512, 512), (1024, 3)):
                    ps1 = psw.tile([C, cn], FP32, name="ln1_ps", tag="w", space="PSUM")
                    mcast = (lambda a: a.bitcast(FP32)) if cn < 256 else (lambda a: a)
                    nc.tensor.matmul(ps1[:, :], mcast(mean96[:, :]),
                                     mcast(x_r[:, c0:c0 + cn]),
                                     start=True, stop=True)
                    nc.scalar.copy(mu_b[:, c0:c0 + cn], ps1[:, :])
                    ps2 = psw.tile([C, cn], FP32, name="ln2_ps", tag="w", space="PSUM")
                    nc.tensor.matmul(ps2[:, :], mcast(mean96[:, :]),
                                     mcast(xsq[:, c0:c0 + cn]),
                                     start=True, stop=True)
                    nc.scalar.copy(msq_b[:, c0:c0 + cn], ps2[:, :])
                istd_b = pre.tile([C, TP], FP32, name="istd_b", tag="istd_b")
                nc.vector.tensor_mul(istd_b[:, :], mu_b[:, :], mu_b[:, :])
                nc.vector.tensor_sub(istd_b[:, :], msq_b[:, :], istd_b[:, :])
                nc.vector.tensor_scalar_add(istd_b[:, :], istd_b[:, :], 1e-5)
                # 1/sqrt(v) = exp(-0.5*ln(v)); Rsqrt/Reciprocal AFs are banned
                nc.scalar.activation(istd_b[:, :], istd_b[:, :], AF.Ln)
                nc.scalar.activation(istd_b[:, :], istd_b[:, :], AF.Exp,
                                     scale=-0.5)
                xn = pre.tile([C, TP], F32R, name="xn", tag="xn")
                nc.vector.tensor_sub(xn[:, :], x_sb[:, :], mu_b[:, :])
                nc.vector.tensor_mul(xn[:, :], xn[:, :].bitcast(FP32), istd_b[:, :])

                # -------- in_proj GEMM (4 M-tiles, base-aligned pieces) ----
                u_raw0 = pre.tile([128, TP], FP32, name="u_raw0", tag="u_raw0")
                u_raw1 = v64(pre, "u_raw1", TP, "u_raw1")
                for c0, cn in ((0, 512), (512, 512), (1024, 3)):
                    if c0 == 0:
                        gw0, gw1, pw = 0, 509, 3
                    elif c0 == 512:
                        gw0, gw1, pw = 509, 1021, 0
                    else:
                        gw0, gw1, pw = 1021, 1024, 0
                    for m in range(4):
                        ps = psw.tile([128, cn], FP32, name="xz_ps", tag="w",
                                      space="PSUM")
                        mcast = ((lambda a: a.bitcast(FP32)) if cn < 256
                                 else (lambda a: a))
                        nc.tensor.matmul(ps[:, :],
                                         mcast(w1T[:, m * 128:(m + 1) * 128]),
                                         mcast(xn[:, c0:c0 + cn]),
                                         start=True, stop=True)
                        if m == 0:
                            nc.scalar.copy(u_raw0[:, c0:c0 + cn], ps[:, :])
                        elif m == 1:
                            nc.scalar.activation(g0[0:64, gw0:gw1], ps[0:64, pw:cn],
                                                 AF.Silu,
                                                 bias=cf("bias_z_a", (0, 64))[:, 0:1])
                            nc.scalar.copy(u_raw1[:, c0:c0 + cn], ps[64:128, :])
                        elif m == 2:
                            nc.scalar.activation(g0[64:128, gw0:gw1],
                                                 ps[64:128, pw:cn], AF.Silu,
                                                 bias=cf("bias_z_a", (64, 128))[:, 0:1])
                        else:
                            nc.scalar.activation(g1[:, gw0:gw1], ps[64:128, pw:cn],
                                                 AF.Silu,
                                                 bias=cf("bias_z_b", (64, 128))[:, 0:1])

                nc.vector.tensor_add(u_raw0[:, 0:3], u_raw0[:, 0:3],
                                     cf("padfix_a")[:, :])
                nc.vector.tensor_add(u_raw1[:, 0:3], u_raw1[:, 0:3],
                                     cf("padfix_b", (64, 128))[:, :])

                # -------- causal conv + SiLU --------
                for (urw, usb, r0, nr, sfx) in ((u_raw0, u0, 0, 128, "a"),
                                                (u_raw1, u1, 128, 64, "b")):
                    cwn = f"convw_{sfx}"
                    bun = f"bias_u_{sfx}"
                    rows = None if nr == 128 else (64, 128)
                    cw = lambda k: cf(cwn, rows)[:, k:k + 1]
                    if nr == 128:
                        tmp = pre.tile([nr, SL], FP32, name=f"cva{r0}", tag="cva",
                                       bufs=2)
                    else:
                        tmp = v64(pre, f"cva{r0}", SL, "cva", bufs=2)
                    nc.vector.tensor_scalar_mul(tmp[:, :], urw[0:nr, 0:SL], cw(0))
                    for kk in (1, 2, 3):
                        nc.vector.scalar_tensor_tensor(
                            tmp[:, :], urw[0:nr, kk:SL + kk], cw(kk), tmp[:, :],
                            OP.mult, OP.add)
                    nc.scalar.activation(usb[:, :], tmp[:, :], AF.Silu,
                                         bias=cf(bun, rows)[:, 0:1])

            if dbg:
                nc.sync.dma_start(dbg_t["dbg_u"][0:128, :], u0f[:, :])
                nc.sync.dma_start(dbg_t["dbg_u"][128:192, :], u1f[:, :])
                gd = wk.tile([128, SL], FP32, name="gd", tag="ydmp", bufs=2)
                nc.scalar.copy(gd[:, :], g0[:, :])
                nc.sync.dma_start(dbg_t["dbg_g"][0:128, :], gd[:, :])
                gd2 = wk.tile([128, SL], FP32, name="gd2", tag="ydmp", bufs=2)
                nc.scalar.copy(gd2[64:128, :], g1[:, :])
                nc.sync.dma_start(dbg_t["dbg_g"][128:192, :], gd2[64:128, :])

            # zeros for the window cumsum scans
            zcw = cst.tile([128, WIN], FP32, name="zcw", tag="zcw")
            nc.vector.memset(zcw[:, :], 0.0)

            # ---------------- per-direction preambles (both hoisted) -------
            # Two passes so the ACT table switches twice (Exp then Ln)
            # instead of thrashing per instruction.
            dirs = ("f", "r")
            dt0_d, dt1_d, dtu0_d, dtu1_d = {}, {}, {}, {}
            brep_d, crep_d, srw0_d, srw1_d, crw_d, fp_d = {}, {}, {}, {}, {}, {}
            pT_d, spt_d = {}, {}
            for w in dirs:
                # xproj GEMM -> pT [70, 1024] (B:0..15, C:32..47, dt:64..69)
                pT = pers.tile([70, SL], F32R, name=f"pT_{w}", tag=f"pT_{w}")
                for c0 in (0, 512):
                    ps = psw.tile([70, 512], FP32, name="pt_ps", tag="w",
                                  space="PSUM")
                    nc.tensor.matmul(ps[:, :],
                                     cfr(f"xprojT_{w}_a")[:, :],
                                     u0[:, c0:c0 + 512],
                                     start=True, stop=False)
                    nc.tensor.matmul(ps[:, :],
                                     cfr(f"xprojT_{w}_b", (64, 128))[:, :],
                                     u1[:, c0:c0 + 512],
                                     start=False, stop=True)
                    nc.scalar.copy(pT[:, c0:c0 + 512], ps[:, :])
                pT_d[w] = pT

                # dt = softplus(dtw @ p_dt + dtb): exp pass here, ln below
                for (po, l0, sfx) in ((0, 0, "a"), (64, 128, "b")):
                    rows = None if po == 0 else (64, 128)
                    for c0 in (0, 512):
                        ps = psw.tile([128, 512], FP32, name="dt_ps", tag="w",
                                      space="PSUM")
                        nc.tensor.matmul(ps[:, :],
                                         cfr(f"dtwT_{w}", (64, 70))[:, l0:l0 + 128],
                                         pT[64:70, c0:c0 + 512],
                                         start=True, stop=True)
                        spt = wk.tile([128, 512], BF16, name="spt",
                                      tag=f"spt_{w}{po}{c0}", bufs=1)
                        nc.scalar.activation(spt[po:128, :], ps[po:128, :], AF.Exp,
                                             bias=cf(f"dtb_{w}_{sfx}", rows)[:, 0:1])
                        spt_d[(w, po, c0)] = spt

            for w in dirs:
                rev = (w == "r")
                pT = pT_d[w]
                dt0 = pers.tile([128, SL], F32R, name=f"dt0_{w}", tag=f"dt0_{w}")
                dt1 = v64(pers, f"dt1_{w}", SL, f"dt1_{w}", dt=F32R)
                for (dst, po) in ((dt0, 0), (dt1, 64)):
                    for c0 in (0, 512):
                        spt = spt_d[(w, po, c0)]
                        nc.scalar.activation(dst[:, c0:c0 + 512],
                                             spt[po:128, :], AF.Ln, bias=1.0)

                # dtu = dt * u
                dtu0 = pers.tile([128, SL], F32R, name=f"dtu0_{w}", tag=f"dtu0_{w}")
                dtu1 = v64(pers, f"dtu1_{w}", SL, f"dtu1_{w}", dt=F32R)
                nc.vector.tensor_mul(dtu0[:, :], dt0[:, :].bitcast(FP32), u0f[:, :])
                nc.vector.tensor_mul(dtu1[:, :], dt1[:, :].bitcast(FP32), u1f[:, :])

                # windowed Srel = inclusive cumsum of dt (direction-aware)
                w0 = 0 if not rev else SL - WIN
                srw0 = pers.tile([128, WIN], BF16, name=f"srw0_{w}", tag=f"srw0_{w}")
                srw1 = v64(pers, f"srw1_{w}", WIN, f"srw1_{w}", dt=BF16)
                for (srct, dstt, nr, po) in ((dt0.bitcast(FP32), srw0, 128, 0),
                                             (dt1.bitcast(FP32), srw1, 64, 64)):
                    zs = zcw[po:po + nr, :]
                    win = srct[0:nr, w0:w0 + WIN]
                    dwin = dstt[0:nr, :]
                    if rev:
                        nc.vector.tensor_tensor_scan(
                            dwin[:, ::-1], win[:, ::-1], zs, 0.0,
                            OP.add, OP.add)
                    else:
                        nc.vector.tensor_tensor_scan(
                            dwin[:, :], win[:, :], zs, 0.0, OP.add, OP.add)

                # B/C replicated to 128 rows
                brep = pers.tile([128, SL], BF16, name=f"brep_{w}", tag=f"brep_{w}")
                crep = pers.tile([128, SL], BF16, name=f"crep_{w}", tag=f"crep_{w}")
                for c0 in (0, 512):
                    psb = psw.tile([128, 512], FP32, name="b_ps", tag="w",
                                   space="PSUM")
                    nc.tensor.matmul(psb[:, :], cfr("oh16s")[0:16, :],
                                     pT[0:16, c0:c0 + 512],
                                     start=True, stop=True)
                    nc.scalar.copy(brep[:, c0:c0 + 512], psb[:, :])
                    psc2 = psw.tile([128, 512], FP32, name="c_ps", tag="w",
                                    space="PSUM")
                    nc.tensor.matmul(psc2[:, :], cfr("oh16s")[32:48, :],
                                     pT[32:48, c0:c0 + 512],
                                     start=True, stop=True)
                    nc.scalar.copy(crep[:, c0:c0 + 512], psc2[:, :])
                crw = pers.tile([128, WIN], BF16, name=f"crw_{w}", tag=f"crw_{w}")
                nc.scalar.copy(crw[:, :], crep[:, w0:w0 + WIN])

                fp_d[w] = pers.tile([128, NT], FP32, name=f"fp_{w}", tag=f"fp_{w}")
                dt0_d[w], dt1_d[w] = dt0, dt1
                dtu0_d[w], dtu1_d[w] = dtu0, dtu1
                brep_d[w], crep_d[w] = brep, crep
                srw0_d[w], srw1_d[w], crw_d[w] = srw0, srw1, crw

            if dbg:
                nc.sync.dma_start(dbg_t["dbg_dt_f"][0:128, :],
                                  dt0_d["f"][:, :].bitcast(FP32))
                nc.sync.dma_start(dbg_t["dbg_dt_f"][128:192, :],
                                  dt1_d["f"][:, :].bitcast(FP32))

            # correction kernels K = exp(A*Srel)*Crep, persistent to post-AG

            K_d = {w: [pers.tile([128, WIN], BF16, name=f"K_{w}_{k}",
                                 tag=f"K_{w}_{k}") for k in range(NT)]
                   for w in dirs}

            # AG buffers (one per direction)
            ag_in = {w: drp.tile([NT, 128], FP32, name=f"ag_in_{w}",
                                 tag=f"ag_in_{w}", space="DRAM") for w in dirs}
            ag_out = {w: drp.tile([NS, NT * 128], FP32, name=f"ag_out_{w}",
                                  tag=f"ag_out_{w}", space="DRAM",
                                  addr_space="Shared") for w in dirs}

            dsb_d = {}

            def emit_carry(w):
                """Post-AllGather work for direction w: neighbor-select the
                carry, accumulate the windowed correction, project to Δ.
                Emitted late so it never head-blocks engine FIFOs on the AG."""
                agc = wk.tile([NS, NT * 128], FP32, name=f"agc_{w}",
                              tag="agc", bufs=1)
                for ch in range(4):
                    cw0 = ch * (NT * 32)
                    nc.sync.dma_start(agc[:, cw0:cw0 + NT * 32],
                                      ag_out[w][:, cw0:cw0 + NT * 32])
                pscb = psc.tile([128, 512], FP32, name=f"pscb_{w}",
                                tag="pscb", space="PSUM")
                hin_ps = pscb[:, 384:384 + NT]
                sel = cf(f"sel_{w}", (0, NS))
                for k in range(NT):
                    nc.tensor.matmul(hin_ps[:, k:k + 1],
                                     agc[:, k * 128:(k + 1) * 128],
                                     sel[:, :], start=True, stop=True)
                h_in = pers.tile([128, NT], FP32, name=f"h_in_{w}",
                                 tag=f"h_in_{w}")
                nc.scalar.copy(h_in[:, :], hin_ps[:, :])
                if dbg:
                    nc.sync.dma_start(dbg_t[f"dbg_hin_{w}"][:, :], h_in[:, :])
                # windowed correction: yc = sum_k red_k(h)^T K_k
                yc = pscb[:, 0:2 * WIN]
                for k in range(NT):
                    r0 = 8 * k
                    t = k if r0 < 128 else k - 8
                    kfirst = 0 if r0 < 128 else 16
                    klast = 15 if r0 < 128 else 23
                    yccol = 0 if r0 < 128 else WIN
                    redh = wk.tile([128, 128], BF16, name="redh", tag="redh",
                                   bufs=3)
                    nc.vector.tensor_scalar_mul(redh[:, :],
                                                cb(f"red128b_{t}")[:, :],
                                                h_in[:, k:k + 1])
                    nc.tensor.matmul(yc[:, yccol:yccol + WIN],
                                     redh[:, :], K_d[w][k][:, :],
                                     start=(k == kfirst), stop=(k == klast))
                if dbg and w == "f":
                    ycd = wk.tile([128, 2 * WIN], FP32, name="ycd", tag="ycd",
                                  bufs=1)
                    nc.scalar.copy(ycd[:, :], yc[:, :])
                    nc.sync.dma_start(dbg_t["dbg_yc_f"][:, :], ycd[:, :])
                # gate + project: Δ = outpT @ (yc * g_win)
                w0 = 0 if w == "f" else SL - WIN
                ycg0 = pers.tile([128, WIN], BF16, name=f"ycg0_{w}",
                                 tag=f"ycg0_{w}")
                ycg1 = v64(pers, f"ycg1_{w}", WIN, f"ycg1_{w}", dt=BF16)
                nc.vector.tensor_mul(ycg0[:, :], yc[:, 0:WIN],
                                     g0[:, w0:w0 + WIN])
                nc.vector.tensor_mul(ycg1[:, :], yc[64:128, WIN:2 * WIN],
                                     g1[:, w0:w0 + WIN])
                dps = psw.tile([C, WIN], FP32, name="dps", tag="w",
                               space="PSUM")
                nc.tensor.matmul(dps[:, :], cb("outpT_a")[:, :], ycg0[:, :],
                                 start=True, stop=False)
                nc.tensor.matmul(dps[:, :], cb("outpT_b", (64, 128))[:, :],
                                 ycg1[:, :], start=False, stop=True)
                dsb = pers.tile([C, WIN], BF16, name=f"dsb_{w}", tag=f"dsb_{w}")
                nc.scalar.copy(dsb[:, :], dps[:, :])
                dsb_d[w] = dsb

            with tc.tile_pool(name="psy", bufs=1, space="PSUM") as psy:
                y_ps0 = psy.tile([128, SL], FP32, name="y_ps0", tag="y0",
                                 space="PSUM")
                y_ps1f = psy.tile([128, SL], FP32, name="y_ps1", tag="y1",
                                  space="PSUM")
                y_ps1 = y_ps1f[64:128]

                for di, w in enumerate(dirs):
                    rev = (w == "r")
                    w0 = 0 if not rev else SL - WIN
                    dt0, dt1 = dt0_d[w], dt1_d[w]
                    dtu0, dtu1 = dtu0_d[w], dtu1_d[w]
                    brep, crep = brep_d[w], crep_d[w]
                    arep = cf(f"Arep_{w}")

                    # ---- per-tile scan pipeline ----
                    for k in range(NT):
                        r0 = 8 * k
                        (srcdt, srcdtu) = (dt0, dtu0) if r0 < 128 else (dt1, dtu1)
                        ro = r0 if r0 < 128 else r0 - 128
                        q0 = (ro // 64) * 64
                        nq = min(64, (128 if r0 < 128 else 64) - q0)
                        oq = q0 if r0 < 128 else 64   # ohs slice base matches rhs
                        v = (ro % 64) // 8
                        ohs = cfr(f"ohs{v}")
                        dA = wk.tile([128, SL], FP32, name="dA", tag="dA", bufs=3)
                        for c0 in (0, 512):
                            rp = psw.tile([128, 512], FP32, name="rep_ps", tag="w",
                                          space="PSUM")
                            nc.tensor.matmul(rp[:, :],
                                             ohs[oq:oq + nq, :],
                                             srcdt[q0:q0 + nq, c0:c0 + 512],
                                             start=True, stop=True)
                            nc.scalar.activation(dA[:, c0:c0 + 512], rp[:, :], AF.Exp,
                                                 scale=arep[:, k:k + 1])
                        dBu = wk.tile([128, SL], FP32, name="dBu", tag="dBu", bufs=3)
                        for c0 in (0, 512):
                            rp = psw.tile([128, 512], FP32, name="rep2_ps", tag="w",
                                          space="PSUM")
                            nc.tensor.matmul(rp[:, :],
                                             ohs[oq:oq + nq, :],
                                             srcdtu[q0:q0 + nq, c0:c0 + 512],
                                             start=True, stop=True)
                            if k % 3 < 2:
                                nc.vector.tensor_mul(dBu[:, c0:c0 + 512], rp[:, :],
                                                     brep[:, c0:c0 + 512])
                            else:
                                # route via ACT + Pool to unload the DVE
                                dtur = wk.tile([128, 512], FP32, name="dtur",
                                               tag="dtur", bufs=3)
                                nc.scalar.copy(dtur[:, :], rp[:, :])
                                nc.gpsimd.tensor_mul(dBu[:, c0:c0 + 512],
                                                     dtur[:, :],
                                                     brep[:, c0:c0 + 512])
                        h = wk.tile([128, SL], FP32, name="h", tag="h", bufs=3)
                        if rev:
                            nc.vector.tensor_tensor_scan(h[:, ::-1], dA[:, ::-1],
                                                         dBu[:, ::-1], 0.0,
                                                         OP.mult, OP.add)
                        else:
                            nc.vector.tensor_tensor_scan(h[:, :], dA[:, :], dBu[:, :],
                                                         0.0, OP.mult, OP.add)
                        fcol = SL - 1 if not rev else 0
                        nc.vector.tensor_copy(fp_d[w][:, k:k + 1],
                                              h[:, fcol:fcol + 1])
                        if dbg and w == "f" and k == 0:
                            nc.sync.dma_start(dbg_t["dbg_h0_f"][:, :], h[:, :])
                        # hC = h * Crep  (Pool engine)
                        hC = wk.tile([128, SL], BF16, name="hC", tag="hC", bufs=2)
                        nc.gpsimd.tensor_mul(hC[:, :], h[:, :], crep[:, :])
                        # y reduce, full-height output (dst base must be 0)
                        yps = y_ps0 if r0 < 128 else y_ps1f
                        t = k if r0 < 128 else k - 8
                        kfirst = 0 if r0 < 128 else 16
                        klast = 15 if r0 < 128 else 23
                        redb = cb(f"red128b_{t}")
                        for c0 in (0, 512):
                            nc.tensor.matmul(yps[:, c0:c0 + 512],
                                             redb[:, :],
                                             hC[:, c0:c0 + 512],
                                             start=(di == 0 and k == kfirst),
                                             stop=(di == 1 and k == klast))

                        # correction kernel K_k = exp(A*Srel_win) * Crep_win
                        src_sr = srw0_d[w] if r0 < 128 else srw1_d[w]
                        srp = psw.tile([128, WIN], FP32, name="srp", tag="w",
                                       space="PSUM")
                        nc.tensor.matmul(srp[:, :], cb(f"ohsb{v}")[oq:oq + nq, :],
                                         src_sr[q0:q0 + nq, 0:WIN],
                                         start=True, stop=True)
                        cpda = wk.tile([128, WIN], BF16, name="cpda", tag="cpda",
                                       bufs=3)
                        nc.scalar.activation(cpda[:, :], srp[:, :], AF.Exp,
                                             scale=arep[:, k:k + 1])
                        nc.vector.tensor_mul(K_d[w][k][:, :], cpda[:, :],
                                             crw_d[w][:, :])

                    # ---- ship finals: transpose fp, DMA, AllGather ----
                    fpt_ps = psw.tile([NT, 128], FP32, name="fpt_ps", tag="w",
                                      space="PSUM")
                    nc.tensor.transpose(fpt_ps[:, :], fp_d[w][:, :],
                                        cf("ident")[:, :])
                    fpt_sb = wk.tile([NT, 128], FP32, name="fpt_sb", tag="fpt",
                                     bufs=2)
                    nc.scalar.copy(fpt_sb[:, :], fpt_ps[:, :])
                    nc.sync.dma_start(ag_in[w][:, :], fpt_sb[:, :])
                    nc.gpsimd.collective_compute(
                        "AllGather", mybir.AluOpType.bypass,
                        ins=[ag_in[w][:, :].opt()],
                        outs=[ag_out[w][:, :].opt()],
                        replica_groups=RG,
                    )

                # f's post-AG work: emitted only now (after the r loop and the
                # r AllGather trigger) so that nothing waiting on AG_f ever
                # sits ahead of r-loop work in an engine FIFO. AG_f has long
                # completed by the time these run; they overlap AG_r instead.
                emit_carry("f")

                # ---- phase A: y_sl*g, out_proj, base output (AG-independent)
                yslg0 = pers.tile([128, SL], BF16, name="yslg0", tag="yslg0")
                yslg1 = v64(pers, "yslg1", SL, "yslg1", dt=BF16)
                nc.vector.scalar_tensor_tensor(yslg0[:, :], u0f[:, :],
                                               cf("Dsum_a")[:, 0:1],
                                               y_ps0[:, :], OP.mult, OP.add)
                nc.vector.scalar_tensor_tensor(yslg1[:, :], u1f[:, :],
                                               cf("Dsum_b", (64, 128))[:, 0:1],
                                               y_ps1[:, :], OP.mult, OP.add)
            # psy closed: y PSUM banks free
            nc.vector.tensor_mul(yslg0[:, :], yslg0[:, :], g0[:, :])
            nc.vector.tensor_mul(yslg1[:, :], yslg1[:, :], g1[:, :])
            if dbg:
                yd = wk.tile([128, SL], FP32, name="yd", tag="ydmp", bufs=2)
                nc.scalar.copy(yd[:, :], yslg0[:, :])
                nc.sync.dma_start(dbg_t["dbg_yslg"][0:128, :], yd[:, :])
                yd2 = wk.tile([128, SL], FP32, name="yd2", tag="ydmp", bufs=2)
                nc.scalar.copy(yd2[64:128, :], yslg1[:, :])
                nc.sync.dma_start(dbg_t["dbg_yslg"][128:192, :], yd2[64:128, :])

            with tc.tile_pool(name="fin", bufs=1) as fnp:
                osl = fnp.tile([C, SL], BF16, name="osl", tag="osl")
                for c0 in (0, 512):
                    ps = psw.tile([C, 512], FP32, name="op_ps", tag="w",
                                  space="PSUM")
                    nc.tensor.matmul(ps[:, :], cb("outpT_a")[:, :],
                                     yslg0[:, c0:c0 + 512],
                                     start=True, stop=False)
                    nc.tensor.matmul(ps[:, :], cb("outpT_b", (64, 128))[:, :],
                                     yslg1[:, c0:c0 + 512],
                                     start=False, stop=True)
                    nc.scalar.copy(osl[:, c0:c0 + 512], ps[:, :])
                if dbg:
                    od = wk.tile([C, SL], FP32, name="od", tag="ydmp", bufs=2)
                    nc.scalar.copy(od[:, :], osl[:, :])
                    nc.sync.dma_start(dbg_t["dbg_osl"][:, :], od[:, :])
                # base (non-window) output: out = osl + x_skip
                MID = SL - 2 * WIN
                fmid = fnp.tile([C, MID], FP32, name="fmid", tag="fmid")
                nc.vector.tensor_add(fmid[:, :], osl[:, WIN:SL - WIN],
                                     x_sb[:, 3 + WIN:3 + SL - WIN])
                nc.sync.dma_start(out_t[:, WIN:SL - WIN], fmid[:, :])

                # r's post-AG work lands here, overlapping phase A above
                emit_carry("r")

                # window finalize per direction
                for w in dirs:
                    w0 = 0 if w == "f" else SL - WIN
                    dsb = dsb_d[w]
                    # s = osq + osl = 2*osl + delta
                    swin = fnp.tile([C, WIN], BF16, name=f"swin_{w}",
                                    tag=f"swin_{w}")
                    nc.vector.scalar_tensor_tensor(swin[:, :], osl[:, w0:w0 + WIN],
                                                   2.0, dsb[:, :],
                                                   OP.mult, OP.add)
                    fps = psw.tile([C, WIN], FP32, name="fps", tag="w",
                                   space="PSUM")
                    nc.tensor.matmul(fps[:, :], cb("fuswT", (0, C))[:, :],
                                     swin[:, :], start=True, stop=True)
                    wgt = fnp.tile([C, WIN], BF16, name=f"wgt_{w}", tag=f"wgt_{w}")
                    nc.scalar.activation(wgt[:, :], fps[:, :], AF.Sigmoid,
                                         bias=cf("fusb", (0, C))[:, 0:1])
                    # out = osl + wgt*delta + skip
                    wd = fnp.tile([C, WIN], BF16, name=f"wd_{w}", tag=f"wd_{w}")
                    nc.vector.tensor_mul(wd[:, :], wgt[:, :], dsb[:, :])
                    o1 = fnp.tile([C, WIN], FP32, name=f"o1_{w}", tag=f"o1_{w}")
                    nc.vector.tensor_add(o1[:, :], wd[:, :], osl[:, w0:w0 + WIN])
                    fwin = fnp.tile([C, WIN], FP32, name=f"fwin_{w}",
                                    tag=f"fwin_{w}")
                    nc.vector.tensor_add(fwin[:, :], o1[:, :],
                                         x_sb[:, 3 + w0:3 + w0 + WIN])
                    nc.sync.dma_start(out_t[:, w0:w0 + WIN], fwin[:, :])

    nc.compile()
    return nc, dbg_t


def _host_prep(inputs):
    """Build per-core input maps (weight folds, const blobs, slices)."""
    import ml_dtypes

    f32 = np.float32
    ln_g = np.asarray(inputs["ln_g"], np.float64)
    ln_b = np.asarray(inputs["ln_b"], np.float64)
    W1 = np.asarray(inputs["in_proj_w"], np.float64)
    W1p = (W1 * ln_g[None, :])
    bW = W1 @ ln_b
    conv_w = np.asarray(inputs["conv_w"], np.float64)
    bias_u = np.asarray(inputs["conv_bias"], np.float64) + bW[:DIN] * conv_w.sum(axis=1)
    bias_z = bW[DIN:]

    x = np.asarray(inputs["x"], np.float32).reshape(C, NS * SL)

    # W1big col layout: [u0..127 | z0..63, u128..191 | pad64, z64..127
    #                    | pad64, z128..191]
    W1big = np.zeros((512, C), np.float64)
    W1big[0:128] = W1p[0:128]
    W1big[128:192] = W1p[DIN:DIN + 64]
    W1big[192:256] = W1p[128:192]
    W1big[320:384] = W1p[DIN + 64:DIN + 128]
    W1big[448:512] = W1p[DIN + 128:DIN + 192]

    def split_ab(vec192):
        """[192(,k)] -> a [128,k] rows 0:128; b [128,k] rows 64:128=128:192."""
        v = np.asarray(vec192, f32)
        if v.ndim == 1:
            v = v[:, None]
        a = np.zeros((128, v.shape[1]), f32)
        b = np.zeros((128, v.shape[1]), f32)
        a[:, :] = v[0:128]
        b[64:128, :] = v[128:192]
        return a, b

    blob = {}
    blobr = {}
    blob["convw_a"], blob["convw_b"] = split_ab(conv_w)
    blob["bias_u_a"], blob["bias_u_b"] = split_ab(bias_u)
    blob["bias_z_a"], blob["bias_z_b"] = split_ab(bias_z)
    Dsum = (np.asarray(inputs["D_f"], np.float64)
            + np.asarray(inputs["D_r"], np.float64))
    blob["Dsum_a"], blob["Dsum_b"] = split_ab(Dsum)
    fusb = np.zeros((128, 1), f32)
    fusb[0:C, 0] = np.asarray(inputs["fus_b"], f32)
    blob["fusb"] = fusb
    blob["ident"] = np.eye(128, dtype=f32)
    oh16s = np.zeros((128, 128), f32)
    for q in range(112):
        for p in range(128):
            if (q % 32) < 16 and p % 16 == q % 32:
                oh16s[q, p] = 1.0
    blobr["oh16s"] = oh16s
    for v in range(8):
        blobr[f"ohs{v}"] = np.asarray(
            [[1.0 if (q % 64) == 8 * v + p // 16 else 0.0
              for p in range(128)] for q in range(128)], f32)
    red = {}
    for t in range(16):
        red[t] = np.asarray(
            [[1.0 if j == 8 * t + p // 16 else 0.0
              for j in range(128)] for p in range(128)], f32)

    for w in ("f", "r"):
        xp = np.asarray(inputs[f"xproj_{w}"], np.float64)   # [38, 192]
        xp70 = np.zeros((70, DIN), np.float64)
        xp70[0:16] = xp[R:R + N]           # B
        xp70[32:48] = xp[R + N:R + 2 * N]  # C
        xp70[64:70] = xp[0:R]              # dt projection
        xpT = np.ascontiguousarray(xp70.T).astype(f32)      # [192, 70]
        a = np.zeros((128, 70), f32)
        b = np.zeros((128, 70), f32)
        a[:, :] = xpT[0:128]
        b[64:128, :] = xpT[128:192]
        blobr[f"xprojT_{w}_a"], blobr[f"xprojT_{w}_b"] = a, b
        dtw70 = np.zeros((128, 256), np.float64)
        dtwt = np.asarray(inputs[f"dt_w_{w}"], np.float64).T   # [6, 192]
        dtw70[64:70, 0:128] = dtwt[:, 0:128]
        dtw70[64:70, 192:256] = dtwt[:, 128:192]
        blobr[f"dtwT_{w}"] = dtw70.astype(f32)
        blob[f"dtb_{w}_a"], blob[f"dtb_{w}_b"] = split_ab(
            np.asarray(inputs[f"dt_b_{w}"], f32))
        A = -np.exp(np.asarray(inputs[f"A_log_{w}"], np.float64))  # [DIN, N]
        arep = np.zeros((128, NT), f32)
        for p in range(128):
            for k in range(NT):
                arep[p, k] = A[8 * k + p // 16, p % 16]
        blob[f"Arep_{w}"] = arep

    bblob = {}
    outpT = np.ascontiguousarray(np.asarray(inputs["out_proj_w"]).T).astype(f32)
    a = np.zeros((128, C), f32)
    b = np.zeros((128, C), f32)
    a[:, :] = outpT[0:128]
    b[64:128, :] = outpT[128:192]
    bblob["outpT_a"], bblob["outpT_b"] = a, b
    fw = np.zeros((128, C), f32)
    fw[0:C, :] = np.ascontiguousarray(np.asarray(inputs["fus_w"]).T).astype(f32)
    bblob["fuswT"] = fw
    for t in range(16):
        bblob[f"red128b_{t}"] = red[t]
    for v in range(8):
        bblob[f"ohsb{v}"] = blobr[f"ohs{v}"]

    shared = {
        "mean96": np.full((C, C), 1.0 / C, f32),
        "w1T": np.ascontiguousarray(W1big.T).astype(f32),
    }

    in_maps = []
    for s in range(NS):
        m = dict(shared)
        xs = np.zeros((C, TP), f32)
        lo = s * SL - 3
        if lo < 0:
            xs[:, 3:] = x[:, 0:SL]
        else:
            xs[:, :] = x[:, lo:(s + 1) * SL]
        m["x_sl"] = xs
        pf = np.zeros((DIN, 3), f32)
        if s == 0:
            pf[:, :] = np.float32(-bW[:DIN, None])
        bl = dict(blob)
        bl["padfix_a"], bl["padfix_b"] = split_ab(pf)
        for w in ("f", "r"):
            j = s - 1 if w == "f" else s + 1
            sel = np.zeros((128, 1), f32)
            if 0 <= j < NS:
                sel[j, 0] = 1.0
            bl[f"sel_{w}"] = sel
        bf = np.zeros((128, F32_COLS), f32)
        for nm, (o, ncol) in _F32_OFF.items():
            bf[:, o:o + ncol] = bl[nm]
        m["blobf"] = bf
        br = np.zeros((128, F32R_COLS), f32)
        for nm, (o, ncol) in _F32R_OFF.items():
            br[:, o:o + ncol] = blobr[nm]
        m["blobr"] = br
        bb = np.zeros((128, BF_COLS), f32)
        for nm, (o, ncol) in _BF_OFF.items():
            bb[:, o:o + ncol] = bblob[nm]
        m["blobb"] = bb.astype(ml_dtypes.bfloat16)
        in_maps.append(m)
    return in_maps


def run_cores(inputs, dbg=False, trace=False):
    from concourse.bass_utils import run_bass_kernel_spmd
    key = ("g", dbg)
    if key not in _cache:
        _cache[key] = _build_graph(dbg=dbg)
    nc, dbg_t = _cache[key]
    in_maps = _host_prep(inputs)
    res = run_bass_kernel_spmd(nc, in_maps, core_ids=list(range(NS)), trace=trace)
    return res, dbg_t


def kernel(**inputs):
    res, _ = run_cores(inputs, dbg=False, trace=False)
    out = np.zeros((C, NS * SL), np.float32)
    for s in range(NS):
        out[:, s * SL:(s + 1) * SL] = res.results[s]["out"]
    return out.reshape(1, C, 8, 32, 32)


# revision 47
# speedup vs baseline: 1.2336x; 1.2336x over previous
"""Trainium2 Bass kernel for nn_AFF_MambaLayer (bi-directional selective scan).

Sharding: one depth-slice (1024 tokens) per core (8 cores). Each core runs
both scan directions for all 192 channels over its slice. Cross-slice decay
is numerically dead (exp(-~0.55*1024)), so the full-sequence scan equals the
local scan plus a WIN-token boundary correction driven by the *neighbor*
core's final state only; finals are exchanged with two small AllGathers.

Partition convention: the 192-channel dimension is split 128 + 64, with the
64-row half stored in [128, *] tiles at base partition 64 (hardware compute
ops require all operands to share a start partition in {0, 32, 64}).
"""
import os
import sys

import numpy as np

sys.path.insert(0, "/opt/trn_rl_repo")

# geometry
C = 96
DIN = 192
N = 16
R = 6
NS = 8           # slices == cores
SL = 1024        # tokens per slice
TP = SL + 3      # with conv left-context
NT = 24          # scan tiles per direction (8 d-channels x 16 states each)
WIN = 128        # correction window (carry decay is dead past ~64)

_cache = {}

# ---- const blob layouts (col offsets), shared host/device ----
_F32_ITEMS = [
    ("padfix_a", 3), ("padfix_b", 3), ("convw_a", 4), ("convw_b", 4),
    ("bias_u_a", 1), ("bias_u_b", 1), ("bias_z_a", 1), ("bias_z_b", 1),
    ("dtb_f_a", 1), ("dtb_f_b", 1), ("dtb_r_a", 1), ("dtb_r_b", 1),
    ("Arep_f", NT), ("Arep_r", NT),
    ("sel_f", 1), ("sel_r", 1),
    ("ident", 128), ("Dsum_a", 1), ("Dsum_b", 1), ("fusb", 1),
]
_F32_OFF = {}
_off = 0
for _nm, _nc in _F32_ITEMS:
    _F32_OFF[_nm] = (_off, _nc)
    _off += _nc
F32_COLS = _off

# f32r blob: constants consumed as matmul stationaries
_F32R_ITEMS = [
    ("oh16s", 128),
    ("xprojT_f_a", 70), ("xprojT_f_b", 70),
    ("xprojT_r_a", 70), ("xprojT_r_b", 70),
    ("dtwT_f", 256), ("dtwT_r", 256),
] + [(f"ohs{v}", 128) for v in range(8)]
_F32R_OFF = {}
_off = 0
for _nm, _nc in _F32R_ITEMS:
    _F32R_OFF[_nm] = (_off, _nc)
    _off += _nc
F32R_COLS = _off

_BF_ITEMS = [("outpT_a", C), ("outpT_b", C), ("fuswT", C)] + [
    (f"red128b_{t}", 128) for t in range(16)] + [
    (f"ohsb{v}", 128) for v in range(8)]
_BF_OFF = {}
_off = 0
for _nm, _nc in _BF_ITEMS:
    _BF_OFF[_nm] = (_off, _nc)
    _off += _nc
BF_COLS = _off


def _build_graph(dbg=False):
    import concourse.bass as bass
    import concourse.bacc as bacc
    import concourse.mybir as mybir
    from concourse import tile

    FP32 = mybir.dt.float32
    F32R = mybir.dt.float32r
    BF16 = mybir.dt.bfloat16
    AF = mybir.ActivationFunctionType
    OP = mybir.AluOpType

    nc = bacc.Bacc("TRN2", target_bir_lowering=False, debug=False, num_devices=NS)

    P = {}

    def inp(name, shape, dt=FP32):
        P[name] = nc.dram_tensor(name, list(shape), dt, kind="ExternalInput").ap()

    inp("x_sl", [C, TP], F32R)
    inp("mean96", [C, C], F32R)
    inp("w1T", [C, 512], F32R)
    inp("blobf", [128, F32_COLS])
    inp("blobr", [128, F32R_COLS], F32R)
    inp("blobb", [128, BF_COLS], BF16)

    out_t = nc.dram_tensor("out", [C, SL], FP32, kind="ExternalOutput").ap()
    dbg_t = {}
    if dbg:
        for name, shape in [
            ("dbg_u", [DIN, SL]), ("dbg_g", [DIN, SL]), ("dbg_dt_f", [DIN, SL]),
            ("dbg_h0_f", [128, SL]), ("dbg_yslg", [DIN, SL]),
            ("dbg_hin_f", [128, NT]), ("dbg_hin_r", [128, NT]),
            ("dbg_yc_f", [128, 2 * WIN]), ("dbg_osl", [C, SL]),
        ]:
            dbg_t[name] = nc.dram_tensor(name, shape, FP32, kind="ExternalOutput").ap()

    RG = [list(range(NS))]

    with tile.TileContext(nc) as tc:
        with (
            tc.tile_pool(name="const", bufs=1) as cst,
            tc.tile_pool(name="pers", bufs=1) as pers,
            tc.tile_pool(name="wk", bufs=3) as wk,
            tc.tile_pool(name="psw", bufs=3, space="PSUM") as psw,
            tc.tile_pool(name="psc", bufs=1, space="PSUM") as psc,
            tc.tile_pool(name="dram", bufs=1, space="DRAM") as drp,
        ):
            # x slice first so LayerNorm can start ASAP
            x_sbr = pers.tile([C, TP], F32R, name="x_sb", tag="x_sb")
            nc.sync.dma_start(x_sbr[:, :], P["x_sl"])
            x_sb = x_sbr.bitcast(FP32)
            mean96 = cst.tile([C, C], F32R, name="mean96", tag="mean96")
            nc.sync.dma_start(mean96[:, :], P["mean96"])
            w1T = cst.tile([C, 512], F32R, name="w1T", tag="w1T")
            nc.sync.dma_start(w1T[:, :], P["w1T"])
            blobf = cst.tile([128, F32_COLS], FP32, name="blobf", tag="blobf")
            nc.sync.dma_start(blobf[:, :], P["blobf"])
            blobr = cst.tile([128, F32R_COLS], F32R, name="blobr", tag="blobr")
            nc.sync.dma_start(blobr[:, :], P["blobr"])
            blobb = cst.tile([128, BF_COLS], BF16, name="blobb", tag="blobb")
            nc.sync.dma_start(blobb[:, :], P["blobb"])

            def cf(nm, rows=None):
                o, ncol = _F32_OFF[nm]
                t = blobf[:, o:o + ncol] if rows is None else \
                    blobf[rows[0]:rows[1], o:o + ncol]
                return t

            def cfr(nm, rows=None):
                o, ncol = _F32R_OFF[nm]
                return blobr[:, o:o + ncol] if rows is None else \
                    blobr[rows[0]:rows[1], o:o + ncol]

            def cb(nm, rows=None):
                o, ncol = _BF_OFF[nm]
                return blobb[:, o:o + ncol] if rows is None else \
                    blobb[rows[0]:rows[1], o:o + ncol]

            def v64(pool, name, cols, tag, bufs=None, dt=FP32):
                """[64, cols] logical tile stored at base partition 64."""
                kw = dict(name=name, tag=tag)
                if bufs is not None:
                    kw["bufs"] = bufs
                t = pool.tile([128, cols], dt, **kw)
                return t[64:128]

            # persistent activations (second halves at base 64)
            g0 = pers.tile([128, SL], BF16, name="g0", tag="g0")
            g1 = v64(pers, "g1", SL, "g1", dt=BF16)
            u0 = pers.tile([128, SL], F32R, name="u0", tag="u0")
            u1 = v64(pers, "u1", SL, "u1", dt=F32R)
            u0f = u0.bitcast(FP32)
            u1f = u1.bitcast(FP32)

            # ---------------- preprocessing (scoped pool) ----------------
            with tc.tile_pool(name="pre", bufs=1) as pre:
                xsq = pre.tile([C, TP], F32R, name="xsq", tag="xsq")
                nc.scalar.square(xsq[:, :], x_sb[:, :])
                x_r = x_sbr
                mu_b = pre.tile([C, TP], FP32, name="mu_b", tag="mu_b")
                msq_b = pre.tile([C, TP], FP32, name="msq_b", tag="msq_b")
                for c0, cn in ((0, 512), (512, 512), (1024, 3)):
                    ps1 = psw.tile([C, cn], FP32, name="ln1_ps", tag="w", space="PSUM")
                    mcast = (lambda a: a.bitcast(FP32)) if cn < 256 else (lambda a: a)
                    nc.tensor.matmul(ps1[:, :], mcast(mean96[:, :]),
                                     mcast(x_r[:, c0:c0 + cn]),
                                     start=True, stop=True)
                    nc.scalar.copy(mu_b[:, c0:c0 + cn], ps1[:, :])
                    ps2 = psw.tile([C, cn], FP32, name="ln2_ps", tag="w", space="PSUM")
                    nc.tensor.matmul(ps2[:, :], mcast(mean96[:, :]),
                                     mcast(xsq[:, c0:c0 + cn]),
                                     start=True, stop=True)
                    nc.scalar.copy(msq_b[:, c0:c0 + cn], ps2[:, :])
                istd_b = pre.tile([C, TP], FP32, name="istd_b", tag="istd_b")
                nc.vector.tensor_mul(istd_b[:, :], mu_b[:, :], mu_b[:, :])
                nc.vector.tensor_sub(istd_b[:, :], msq_b[:, :], istd_b[:, :])
                nc.vector.tensor_scalar_add(istd_b[:, :], istd_b[:, :], 1e-5)
                # 1/sqrt(v) = exp(-0.5*ln(v)); Rsqrt/Reciprocal AFs are banned
                nc.scalar.activation(istd_b[:, :], istd_b[:, :], AF.Ln)
                nc.scalar.activation(istd_b[:, :], istd_b[:, :], AF.Exp,
                                     scale=-0.5)
                xn = pre.tile([C, TP], F32R, name="xn", tag="xn")
                nc.vector.tensor_sub(xn[:, :], x_sb[:, :], mu_b[:, :])
                nc.vector.tensor_mul(xn[:, :], xn[:, :].bitcast(FP32), istd_b[:, :])

                # -------- in_proj GEMM (4 M-tiles, base-aligned pieces) ----
                u_raw0 = pre.tile([128, TP], FP32, name="u_raw0", tag="u_raw0")
                u_raw1 = v64(pre, "u_raw1", TP, "u_raw1")
                for c0, cn in ((0, 512), (512, 512), (1024, 3)):
                    if c0 == 0:
                        gw0, gw1, pw = 0, 509, 3
                    elif c0 == 512:
                        gw0, gw1, pw = 509, 1021, 0
                    else:
                        gw0, gw1, pw = 1021, 1024, 0
                    for m in range(4):
                        ps = psw.tile([128, cn], FP32, name="xz_ps", tag="w",
                                      space="PSUM")
                        mcast = ((lambda a: a.bitcast(FP32)) if cn < 256
                                 else (lambda a: a))
                        nc.tensor.matmul(ps[:, :],
                                         mcast(w1T[:, m * 128:(m + 1) * 128]),
                                         mcast(xn[:, c0:c0 + cn]),
                                         start=True, stop=True)
                        if m == 0:
                            nc.scalar.copy(u_raw0[:, c0:c0 + cn], ps[:, :])
                        elif m == 1:
                            nc.scalar.activation(g0[0:64, gw0:gw1], ps[0:64, pw:cn],
                                                 AF.Silu,
                                                 bias=cf("bias_z_a", (0, 64))[:, 0:1])
                            nc.scalar.copy(u_raw1[:, c0:c0 + cn], ps[64:128, :])
                        elif m == 2:
                            nc.scalar.activation(g0[64:128, gw0:gw1],
                                                 ps[64:128, pw:cn], AF.Silu,
                                                 bias=cf("bias_z_a", (64, 128))[:, 0:1])
                        else:
                            nc.scalar.activation(g1[:, gw0:gw1], ps[64:128, pw:cn],
                                                 AF.Silu,
                                                 bias=cf("bias_z_b", (64, 128))[:, 0:1])

                nc.vector.tensor_add(u_raw0[:, 0:3], u_raw0[:, 0:3],
                                     cf("padfix_a")[:, :])
                nc.vector.tensor_add(u_raw1[:, 0:3], u_raw1[:, 0:3],
                                     cf("padfix_b", (64, 128))[:, :])

                # -------- causal conv + SiLU --------
                for (urw, usb, r0, nr, sfx) in ((u_raw0, u0, 0, 128, "a"),
                                                (u_raw1, u1, 128, 64, "b")):
                    cwn = f"convw_{sfx}"
                    bun = f"bias_u_{sfx}"
                    rows = None if nr == 128 else (64, 128)
                    cw = lambda k: cf(cwn, rows)[:, k:k + 1]
                    if nr == 128:
                        tmp = pre.tile([nr, SL], FP32, name=f"cva{r0}", tag="cva",
                                       bufs=2)
                    else:
                        tmp = v64(pre, f"cva{r0}", SL, "cva", bufs=2)
                    nc.vector.tensor_scalar_mul(tmp[:, :], urw[0:nr, 0:SL], cw(0))
                    for kk in (1, 2, 3):
                        nc.vector.scalar_tensor_tensor(
                            tmp[:, :], urw[0:nr, kk:SL + kk], cw(kk), tmp[:, :],
                            OP.mult, OP.add)
                    nc.scalar.activation(usb[:, :], tmp[:, :], AF.Silu,
                                         bias=cf(bun, rows)[:, 0:1])

            if dbg:
                nc.sync.dma_start(dbg_t["dbg_u"][0:128, :], u0f[:, :])
                nc.sync.dma_start(dbg_t["dbg_u"][128:192, :], u1f[:, :])
                gd = wk.tile([128, SL], FP32, name="gd", tag="ydmp", bufs=2)
                nc.scalar.copy(gd[:, :], g0[:, :])
                nc.sync.dma_start(dbg_t["dbg_g"][0:128, :], gd[:, :])
                gd2 = wk.tile([128, SL], FP32, name="gd2", tag="ydmp", bufs=2)
                nc.scalar.copy(gd2[64:128, :], g1[:, :])
                nc.sync.dma_start(dbg_t["dbg_g"][128:192, :], gd2[64:128, :])

            # zeros for the window cumsum scans
            zcw = cst.tile([128, WIN], FP32, name="zcw", tag="zcw")
            nc.vector.memset(zcw[:, :], 0.0)

            # ---------------- per-direction preambles (both hoisted) -------
            # Two passes so the ACT table switches twice (Exp then Ln)
            # instead of thrashing per instruction.
            dirs = ("f", "r")
            dt0_d, dt1_d, dtu0_d, dtu1_d = {}, {}, {}, {}
            brep_d, crep_d, srw0_d, srw1_d, crw_d, fp_d = {}, {}, {}, {}, {}, {}
            pT_d, spt_d = {}, {}
            for w in dirs:
                # xproj GEMM -> pT [70, 1024] (B:0..15, C:32..47, dt:64..69)
                pT = pers.tile([70, SL], F32R, name=f"pT_{w}", tag=f"pT_{w}")
                for c0 in (0, 512):
                    ps = psw.tile([70, 512], FP32, name="pt_ps", tag="w",
                                  space="PSUM")
                    nc.tensor.matmul(ps[:, :],
                                     cfr(f"xprojT_{w}_a")[:, :],
                                     u0[:, c0:c0 + 512],
                                     start=True, stop=False)
                    nc.tensor.matmul(ps[:, :],
                                     cfr(f"xprojT_{w}_b", (64, 128))[:, :],
                                     u1[:, c0:c0 + 512],
                                     start=False, stop=True)
                    nc.scalar.copy(pT[:, c0:c0 + 512], ps[:, :])
                pT_d[w] = pT

                # dt = softplus(dtw @ p_dt + dtb): exp pass here, ln below
                for (po, l0, sfx) in ((0, 0, "a"), (64, 128, "b")):
                    rows = None if po == 0 else (64, 128)
                    for c0 in (0, 512):
                        ps = psw.tile([128, 512], FP32, name="dt_ps", tag="w",
                                      space="PSUM")
                        nc.tensor.matmul(ps[:, :],
                                         cfr(f"dtwT_{w}", (64, 70))[:, l0:l0 + 128],
                                         pT[64:70, c0:c0 + 512],
                                         start=True, stop=True)
                        spt = wk.tile([128, 512], BF16, name="spt",
                                      tag=f"spt_{w}{po}{c0}", bufs=1)
                        nc.scalar.activation(spt[po:128, :], ps[po:128, :], AF.Exp,
                                             bias=cf(f"dtb_{w}_{sfx}", rows)[:, 0:1])
                        spt_d[(w, po, c0)] = spt

            for w in dirs:
                rev = (w == "r")
                pT = pT_d[w]
                dt0 = pers.tile([128, SL], F32R, name=f"dt0_{w}", tag=f"dt0_{w}")
                dt1 = v64(pers, f"dt1_{w}", SL, f"dt1_{w}", dt=F32R)
                for (dst, po) in ((dt0, 0), (dt1, 64)):
                    for c0 in (0, 512):
                        spt = spt_d[(w, po, c0)]
                        nc.scalar.activation(dst[:, c0:c0 + 512],
                                             spt[po:128, :], AF.Ln, bias=1.0)

                # dtu = dt * u
                dtu0 = pers.tile([128, SL], F32R, name=f"dtu0_{w}", tag=f"dtu0_{w}")
                dtu1 = v64(pers, f"dtu1_{w}", SL, f"dtu1_{w}", dt=F32R)
                nc.vector.tensor_mul(dtu0[:, :], dt0[:, :].bitcast(FP32), u0f[:, :])
                nc.vector.tensor_mul(dtu1[:, :], dt1[:, :].bitcast(FP32), u1f[:, :])

                # windowed Srel = inclusive cumsum of dt (direction-aware)
                w0 = 0 if not rev else SL - WIN
                srw0 = pers.tile([128, WIN], BF16, name=f"srw0_{w}", tag=f"srw0_{w}")
                srw1 = v64(pers, f"srw1_{w}", WIN, f"srw1_{w}", dt=BF16)
                for (srct, dstt, nr, po) in ((dt0.bitcast(FP32), srw0, 128, 0),
                                             (dt1.bitcast(FP32), srw1, 64, 64)):
                    zs = zcw[po:po + nr, :]
                    win = srct[0:nr, w0:w0 + WIN]
                    dwin = dstt[0:nr, :]
                    if rev:
                        nc.vector.tensor_tensor_scan(
                            dwin[:, ::-1], win[:, ::-1], zs, 0.0,
                            OP.add, OP.add)
                    else:
                        nc.vector.tensor_tensor_scan(
                            dwin[:, :], win[:, :], zs, 0.0, OP.add, OP.add)

                # B/C replicated to 128 rows
                brep = pers.tile([128, SL], BF16, name=f"brep_{w}", tag=f"brep_{w}")
                crep = pers.tile([128, SL], BF16, name=f"crep_{w}", tag=f"crep_{w}")
                for c0 in (0, 512):
                    psb = psw.tile([128, 512], FP32, name="b_ps", tag="w",
                                   space="PSUM")
                    nc.tensor.matmul(psb[:, :], cfr("oh16s")[0:16, :],
                                     pT[0:16, c0:c0 + 512],
                                     start=True, stop=True)
                    nc.scalar.copy(brep[:, c0:c0 + 512], psb[:, :])
                    psc2 = psw.tile([128, 512], FP32, name="c_ps", tag="w",
                                    space="PSUM")
                    nc.tensor.matmul(psc2[:, :], cfr("oh16s")[32:48, :],
                                     pT[32:48, c0:c0 + 512],
                                     start=True, stop=True)
                    nc.scalar.copy(crep[:, c0:c0 + 512], psc2[:, :])
                crw = pers.tile([128, WIN], BF16, name=f"crw_{w}", tag=f"crw_{w}")
                nc.scalar.copy(crw[:, :], crep[:, w0:w0 + WIN])

                fp_d[w] = pers.tile([128, NT], FP32, name=f"fp_{w}", tag=f"fp_{w}")
                dt0_d[w], dt1_d[w] = dt0, dt1
                dtu0_d[w], dtu1_d[w] = dtu0, dtu1
                brep_d[w], crep_d[w] = brep, crep
                srw0_d[w], srw1_d[w], crw_d[w] = srw0, srw1, crw

            if dbg:
                nc.sync.dma_start(dbg_t["dbg_dt_f"][0:128, :],
                                  dt0_d["f"][:, :].bitcast(FP32))
                nc.sync.dma_start(dbg_t["dbg_dt_f"][128:192, :],
                                  dt1_d["f"][:, :].bitcast(FP32))

            # correction kernels K = exp(A*Srel)*Crep, persistent to post-AG

            K_d = {w: [pers.tile([128, WIN], BF16, name=f"K_{w}_{k}",
                                 tag=f"K_{w}_{k}") for k in range(NT)]
                   for w in dirs}

            # AG buffers (one per direction)
            ag_in = {w: drp.tile([NT, 128], FP32, name=f"ag_in_{w}",
                                 tag=f"ag_in_{w}", space="DRAM") for w in dirs}
            ag_out = {w: drp.tile([NS, NT * 128], FP32, name=f"ag_out_{w}",
                                  tag=f"ag_out_{w}", space="DRAM",
                                  addr_space="Shared") for w in dirs}

            # Tiny startup collective: absorbs the inter-core launch skew
            # early (while only DMAs/preproc run), so the two real
            # AllGathers later see aligned cores and complete fast.
            sync_in = drp.tile([1, 8], FP32, name="sync_in", tag="sync_in",
                               space="DRAM")
            sync_out = drp.tile([NS, 8], FP32, name="sync_out", tag="sync_out",
                                space="DRAM", addr_space="Shared")
            sync_sb = wk.tile([1, 8], FP32, name="sync_sb", tag="sync_sb",
                              bufs=1)
            nc.vector.memset(sync_sb[:, :], 0.0)
            nc.sync.dma_start(sync_in[:, :], sync_sb[:, :])
            nc.gpsimd.collective_compute(
                "AllGather", mybir.AluOpType.bypass,
                ins=[sync_in[:, :].opt()],
                outs=[sync_out[:, :].opt()],
                replica_groups=RG,
            )

            dsb_d = {}

            def emit_carry(w):
                """Post-AllGather work for direction w: neighbor-select the
                carry, accumulate the windowed correction, project to Δ.
                Emitted late so it never head-blocks engine FIFOs on the AG."""
                agc = wk.tile([NS, NT * 128], FP32, name=f"agc_{w}",
                              tag="agc", bufs=1)
                for ch in range(4):
                    cw0 = ch * (NT * 32)
                    nc.sync.dma_start(agc[:, cw0:cw0 + NT * 32],
                                      ag_out[w][:, cw0:cw0 + NT * 32])
                pscb = psc.tile([128, 512], FP32, name=f"pscb_{w}",
                                tag="pscb", space="PSUM")
                hin_ps = pscb[:, 384:384 + NT]
                sel = cf(f"sel_{w}", (0, NS))
                for k in range(NT):
                    nc.tensor.matmul(hin_ps[:, k:k + 1],
                                     agc[:, k * 128:(k + 1) * 128],
                                     sel[:, :], start=True, stop=True)
                h_in = pers.tile([128, NT], FP32, name=f"h_in_{w}",
                                 tag=f"h_in_{w}")
                nc.scalar.copy(h_in[:, :], hin_ps[:, :])
                if dbg:
                    nc.sync.dma_start(dbg_t[f"dbg_hin_{w}"][:, :], h_in[:, :])
                # windowed correction: yc = sum_k red_k(h)^T K_k
                yc = pscb[:, 0:2 * WIN]
                for k in range(NT):
                    r0 = 8 * k
                    t = k if r0 < 128 else k - 8
                    kfirst = 0 if r0 < 128 else 16
                    klast = 15 if r0 < 128 else 23
                    yccol = 0 if r0 < 128 else WIN
                    redh = wk.tile([128, 128], BF16, name="redh", tag="redh",
                                   bufs=3)
                    nc.vector.tensor_scalar_mul(redh[:, :],
                                                cb(f"red128b_{t}")[:, :],
                                                h_in[:, k:k + 1])
                    nc.tensor.matmul(yc[:, yccol:yccol + WIN],
                                     redh[:, :], K_d[w][k][:, :],
                                     start=(k == kfirst), stop=(k == klast))
                if dbg and w == "f":
                    ycd = wk.tile([128, 2 * WIN], FP32, name="ycd", tag="ycd",
                                  bufs=1)
                    nc.scalar.copy(ycd[:, :], yc[:, :])
                    nc.sync.dma_start(dbg_t["dbg_yc_f"][:, :], ycd[:, :])
                # gate + project: Δ = outpT @ (yc * g_win)
                w0 = 0 if w == "f" else SL - WIN
                ycg0 = pers.tile([128, WIN], BF16, name=f"ycg0_{w}",
                                 tag=f"ycg0_{w}")
                ycg1 = v64(pers, f"ycg1_{w}", WIN, f"ycg1_{w}", dt=BF16)
                nc.vector.tensor_mul(ycg0[:, :], yc[:, 0:WIN],
                                     g0[:, w0:w0 + WIN])
                nc.vector.tensor_mul(ycg1[:, :], yc[64:128, WIN:2 * WIN],
                                     g1[:, w0:w0 + WIN])
                dps = psw.tile([C, WIN], FP32, name="dps", tag="w",
                               space="PSUM")
                nc.tensor.matmul(dps[:, :], cb("outpT_a")[:, :], ycg0[:, :],
                                 start=True, stop=False)
                nc.tensor.matmul(dps[:, :], cb("outpT_b", (64, 128))[:, :],
                                 ycg1[:, :], start=False, stop=True)
                dsb = pers.tile([C, WIN], BF16, name=f"dsb_{w}", tag=f"dsb_{w}")
                nc.scalar.copy(dsb[:, :], dps[:, :])
                dsb_d[w] = dsb

            with tc.tile_pool(name="psy", bufs=1, space="PSUM") as psy:
                y_ps0 = psy.tile([128, SL], FP32, name="y_ps0", tag="y0",
                                 space="PSUM")
                y_ps1f = psy.tile([128, SL], FP32, name="y_ps1", tag="y1",
                                  space="PSUM")
                y_ps1 = y_ps1f[64:128]

                for di, w in enumerate(dirs):
                    rev = (w == "r")
                    w0 = 0 if not rev else SL - WIN
                    dt0, dt1 = dt0_d[w], dt1_d[w]
                    dtu0, dtu1 = dtu0_d[w], dtu1_d[w]
                    brep, crep = brep_d[w], crep_d[w]
                    arep = cf(f"Arep_{w}")

                    # ---- per-tile scan pipeline ----
                    for k in range(NT):
                        r0 = 8 * k
                        (srcdt, srcdtu) = (dt0, dtu0) if r0 < 128 else (dt1, dtu1)
                        ro = r0 if r0 < 128 else r0 - 128
                        q0 = (ro // 64) * 64
                        nq = min(64, (128 if r0 < 128 else 64) - q0)
                        oq = q0 if r0 < 128 else 64   # ohs slice base matches rhs
                        v = (ro % 64) // 8
                        ohs = cfr(f"ohs{v}")
                        dA = wk.tile([128, SL], FP32, name="dA", tag="dA", bufs=3)
                        for c0 in (0, 512):
                            rp = psw.tile([128, 512], FP32, name="rep_ps", tag="w",
                                          space="PSUM")
                            nc.tensor.matmul(rp[:, :],
                                             ohs[oq:oq + nq, :],
                                             srcdt[q0:q0 + nq, c0:c0 + 512],
                                             start=True, stop=True)
                            nc.scalar.activation(dA[:, c0:c0 + 512], rp[:, :], AF.Exp,
                                                 scale=arep[:, k:k + 1])
                        dBu = wk.tile([128, SL], FP32, name="dBu", tag="dBu", bufs=3)
                        for c0 in (0, 512):
                            rp = psw.tile([128, 512], FP32, name="rep2_ps", tag="w",
                                          space="PSUM")
                            nc.tensor.matmul(rp[:, :],
                                             ohs[oq:oq + nq, :],
                                             srcdtu[q0:q0 + nq, c0:c0 + 512],
                                             start=True, stop=True)
                            if k % 3 < 2:
                                nc.vector.tensor_mul(dBu[:, c0:c0 + 512], rp[:, :],
                                                     brep[:, c0:c0 + 512])
                            else:
                                # route via ACT + Pool to unload the DVE
                                dtur = wk.tile([128, 512], FP32, name="dtur",
                                               tag="dtur", bufs=3)
                                nc.scalar.copy(dtur[:, :], rp[:, :])
                                nc.gpsimd.tensor_mul(dBu[:, c0:c0 + 512],
                                                     dtur[:, :],
                                                     brep[:, c0:c0 + 512])
                        h = wk.tile([128, SL], FP32, name="h", tag="h", bufs=3)
                        if rev:
                            nc.vector.tensor_tensor_scan(h[:, ::-1], dA[:, ::-1],
                                                         dBu[:, ::-1], 0.0,
                                                         OP.mult, OP.add)
                        else:
                            nc.vector.tensor_tensor_scan(h[:, :], dA[:, :], dBu[:, :],
                                                         0.0, OP.mult, OP.add)
                        fcol = SL - 1 if not rev else 0
                        nc.vector.tensor_copy(fp_d[w][:, k:k + 1],
                                              h[:, fcol:fcol + 1])
                        if dbg and w == "f" and k == 0:
                            nc.sync.dma_start(dbg_t["dbg_h0_f"][:, :], h[:, :])
                        # hC = h * Crep  (Pool engine)
                        hC = wk.tile([128, SL], BF16, name="hC", tag="hC", bufs=2)
                        nc.gpsimd.tensor_mul(hC[:, :], h[:, :], crep[:, :])
                        # y reduce, full-height output (dst base must be 0)
                        yps = y_ps0 if r0 < 128 else y_ps1f
                        t = k if r0 < 128 else k - 8
                        kfirst = 0 if r0 < 128 else 16
                        klast = 15 if r0 < 128 else 23
                        redb = cb(f"red128b_{t}")
                        for c0 in (0, 512):
                            nc.tensor.matmul(yps[:, c0:c0 + 512],
                                             redb[:, :],
                                             hC[:, c0:c0 + 512],
                                             start=(di == 0 and k == kfirst),
                                             stop=(di == 1 and k == klast))

                        # correction kernel K_k = exp(A*Srel_win) * Crep_win
                        src_sr = srw0_d[w] if r0 < 128 else srw1_d[w]
                        srp = psw.tile([128, WIN], FP32, name="srp", tag="w",
                                       space="PSUM")
                        nc.tensor.matmul(srp[:, :], cb(f"ohsb{v}")[oq:oq + nq, :],
                                         src_sr[q0:q0 + nq, 0:WIN],
                                         start=True, stop=True)
                        cpda = wk.tile([128, WIN], BF16, name="cpda", tag="cpda",
                                       bufs=3)
                        nc.scalar.activation(cpda[:, :], srp[:, :], AF.Exp,
                                             scale=arep[:, k:k + 1])
                        nc.vector.tensor_mul(K_d[w][k][:, :], cpda[:, :],
                                             crw_d[w][:, :])

                    # ---- ship finals: transpose fp, DMA, AllGather ----
                    fpt_ps = psw.tile([NT, 128], FP32, name="fpt_ps", tag="w",
                                      space="PSUM")
                    nc.tensor.transpose(fpt_ps[:, :], fp_d[w][:, :],
                                        cf("ident")[:, :])
                    fpt_sb = wk.tile([NT, 128], FP32, name="fpt_sb", tag="fpt",
                                     bufs=2)
                    nc.scalar.copy(fpt_sb[:, :], fpt_ps[:, :])
                    nc.sync.dma_start(ag_in[w][:, :], fpt_sb[:, :])
                    nc.gpsimd.collective_compute(
                        "AllGather", mybir.AluOpType.bypass,
                        ins=[ag_in[w][:, :].opt()],
                        outs=[ag_out[w][:, :].opt()],
                        replica_groups=RG,
                    )

                # f's post-AG work: wait-stamped far into simulated time so
                # the list scheduler cannot hoist anything AG_f-gated ahead
                # of r-loop work in an engine's program (which stalls that
                # engine for the AllGather's full latency).
                with tc.tile_wait_until(0.8):
                    emit_carry("f")

                # ---- phase A: y_sl*g, out_proj, base output (AG-independent)
                yslg0 = pers.tile([128, SL], BF16, name="yslg0", tag="yslg0")
                yslg1 = v64(pers, "yslg1", SL, "yslg1", dt=BF16)
                nc.vector.scalar_tensor_tensor(yslg0[:, :], u0f[:, :],
                                               cf("Dsum_a")[:, 0:1],
                                               y_ps0[:, :], OP.mult, OP.add)
                nc.vector.scalar_tensor_tensor(yslg1[:, :], u1f[:, :],
                                               cf("Dsum_b", (64, 128))[:, 0:1],
                                               y_ps1[:, :], OP.mult, OP.add)
            # psy closed: y PSUM banks free
            nc.vector.tensor_mul(yslg0[:, :], yslg0[:, :], g0[:, :])
            nc.vector.tensor_mul(yslg1[:, :], yslg1[:, :], g1[:, :])
            if dbg:
                yd = wk.tile([128, SL], FP32, name="yd", tag="ydmp", bufs=2)
                nc.scalar.copy(yd[:, :], yslg0[:, :])
                nc.sync.dma_start(dbg_t["dbg_yslg"][0:128, :], yd[:, :])
                yd2 = wk.tile([128, SL], FP32, name="yd2", tag="ydmp", bufs=2)
                nc.scalar.copy(yd2[64:128, :], yslg1[:, :])
                nc.sync.dma_start(dbg_t["dbg_yslg"][128:192, :], yd2[64:128, :])

            with tc.tile_pool(name="fin", bufs=1) as fnp:
                osl = fnp.tile([C, SL], BF16, name="osl", tag="osl")
                for c0 in (0, 512):
                    ps = psw.tile([C, 512], FP32, name="op_ps", tag="w",
                                  space="PSUM")
                    nc.tensor.matmul(ps[:, :], cb("outpT_a")[:, :],
                                     yslg0[:, c0:c0 + 512],
                                     start=True, stop=False)
                    nc.tensor.matmul(ps[:, :], cb("outpT_b", (64, 128))[:, :],
                                     yslg1[:, c0:c0 + 512],
                                     start=False, stop=True)
                    nc.scalar.copy(osl[:, c0:c0 + 512], ps[:, :])
                if dbg:
                    od = wk.tile([C, SL], FP32, name="od", tag="ydmp", bufs=2)
                    nc.scalar.copy(od[:, :], osl[:, :])
                    nc.sync.dma_start(dbg_t["dbg_osl"][:, :], od[:, :])
                # base (non-window) output: out = osl + x_skip
                MID = SL - 2 * WIN
                fmid = fnp.tile([C, MID], FP32, name="fmid", tag="fmid")
                nc.vector.tensor_add(fmid[:, :], osl[:, WIN:SL - WIN],
                                     x_sb[:, 3 + WIN:3 + SL - WIN])
                nc.sync.dma_start(out_t[:, WIN:SL - WIN], fmid[:, :])

                # r's post-AG work lands here, overlapping phase A above
                with tc.tile_wait_until(0.85):
                    emit_carry("r")

                # window finalize per direction
                for w in dirs:
                    w0 = 0 if w == "f" else SL - WIN
                    dsb = dsb_d[w]
                    # s = osq + osl = 2*osl + delta
                    swin = fnp.tile([C, WIN], BF16, name=f"swin_{w}",
                                    tag=f"swin_{w}")
                    nc.vector.scalar_tensor_tensor(swin[:, :], osl[:, w0:w0 + WIN],
                                                   2.0, dsb[:, :],
                                                   OP.mult, OP.add)
                    fps = psw.tile([C, WIN], FP32, name="fps", tag="w",
                                   space="PSUM")
                    nc.tensor.matmul(fps[:, :], cb("fuswT", (0, C))[:, :],
                                     swin[:, :], start=True, stop=True)
                    wgt = fnp.tile([C, WIN], BF16, name=f"wgt_{w}", tag=f"wgt_{w}")
                    nc.scalar.activation(wgt[:, :], fps[:, :], AF.Sigmoid,
                                         bias=cf("fusb", (0, C))[:, 0:1])
                    # out = osl + wgt*delta + skip
                    wd = fnp.tile([C, WIN], BF16, name=f"wd_{w}", tag=f"wd_{w}")
                    nc.vector.tensor_mul(wd[:, :], wgt[:, :], dsb[:, :])
                    o1 = fnp.tile([C, WIN], FP32, name=f"o1_{w}", tag=f"o1_{w}")
                    nc.vector.tensor_add(o1[:, :], wd[:, :], osl[:, w0:w0 + WIN])
                    fwin = fnp.tile([C, WIN], FP32, name=f"fwin_{w}",
                                    tag=f"fwin_{w}")
                    nc.vector.tensor_add(fwin[:, :], o1[:, :],
                                         x_sb[:, 3 + w0:3 + w0 + WIN])
                    nc.sync.dma_start(out_t[:, w0:w0 + WIN], fwin[:, :])

    nc.compile()
    return nc, dbg_t


def _host_prep(inputs):
    """Build per-core input maps (weight folds, const blobs, slices)."""
    import ml_dtypes

    f32 = np.float32
    ln_g = np.asarray(inputs["ln_g"], np.float64)
    ln_b = np.asarray(inputs["ln_b"], np.float64)
    W1 = np.asarray(inputs["in_proj_w"], np.float64)
    W1p = (W1 * ln_g[None, :])
    bW = W1 @ ln_b
    conv_w = np.asarray(inputs["conv_w"], np.float64)
    bias_u = np.asarray(inputs["conv_bias"], np.float64) + bW[:DIN] * conv_w.sum(axis=1)
    bias_z = bW[DIN:]

    x = np.asarray(inputs["x"], np.float32).reshape(C, NS * SL)

    # W1big col layout: [u0..127 | z0..63, u128..191 | pad64, z64..127
    #                    | pad64, z128..191]
    W1big = np.zeros((512, C), np.float64)
    W1big[0:128] = W1p[0:128]
    W1big[128:192] = W1p[DIN:DIN + 64]
    W1big[192:256] = W1p[128:192]
    W1big[320:384] = W1p[DIN + 64:DIN + 128]
    W1big[448:512] = W1p[DIN + 128:DIN + 192]

    def split_ab(vec192):
        """[192(,k)] -> a [128,k] rows 0:128; b [128,k] rows 64:128=128:192."""
        v = np.asarray(vec192, f32)
        if v.ndim == 1:
            v = v[:, None]
        a = np.zeros((128, v.shape[1]), f32)
        b = np.zeros((128, v.shape[1]), f32)
        a[:, :] = v[0:128]
        b[64:128, :] = v[128:192]
        return a, b

    blob = {}
    blobr = {}
    blob["convw_a"], blob["convw_b"] = split_ab(conv_w)
    blob["bias_u_a"], blob["bias_u_b"] = split_ab(bias_u)
    blob["bias_z_a"], blob["bias_z_b"] = split_ab(bias_z)
    Dsum = (np.asarray(inputs["D_f"], np.float64)
            + np.asarray(inputs["D_r"], np.float64))
    blob["Dsum_a"], blob["Dsum_b"] = split_ab(Dsum)
    fusb = np.zeros((128, 1), f32)
    fusb[0:C, 0] = np.asarray(inputs["fus_b"], f32)
    blob["fusb"] = fusb
    blob["ident"] = np.eye(128, dtype=f32)
    oh16s = np.zeros((128, 128), f32)
    for q in range(112):
        for p in range(128):
            if (q % 32) < 16 and p % 16 == q % 32:
                oh16s[q, p] = 1.0
    blobr["oh16s"] = oh16s
    for v in range(8):
        blobr[f"ohs{v}"] = np.asarray(
            [[1.0 if (q % 64) == 8 * v + p // 16 else 0.0
              for p in range(128)] for q in range(128)], f32)
    red = {}
    for t in range(16):
        red[t] = np.asarray(
            [[1.0 if j == 8 * t + p // 16 else 0.0
              for j in range(128)] for p in range(128)], f32)

    for w in ("f", "r"):
        xp = np.asarray(inputs[f"xproj_{w}"], np.float64)   # [38, 192]
        xp70 = np.zeros((70, DIN), np.float64)
        xp70[0:16] = xp[R:R + N]           # B
        xp70[32:48] = xp[R + N:R + 2 * N]  # C
        xp70[64:70] = xp[0:R]              # dt projection
        xpT = np.ascontiguousarray(xp70.T).astype(f32)      # [192, 70]
        a = np.zeros((128, 70), f32)
        b = np.zeros((128, 70), f32)
        a[:, :] = xpT[0:128]
        b[64:128, :] = xpT[128:192]
        blobr[f"xprojT_{w}_a"], blobr[f"xprojT_{w}_b"] = a, b
        dtw70 = np.zeros((128, 256), np.float64)
        dtwt = np.asarray(inputs[f"dt_w_{w}"], np.float64).T   # [6, 192]
        dtw70[64:70, 0:128] = dtwt[:, 0:128]
        dtw70[64:70, 192:256] = dtwt[:, 128:192]
        blobr[f"dtwT_{w}"] = dtw70.astype(f32)
        blob[f"dtb_{w}_a"], blob[f"dtb_{w}_b"] = split_ab(
            np.asarray(inputs[f"dt_b_{w}"], f32))
        A = -np.exp(np.asarray(inputs[f"A_log_{w}"], np.float64))  # [DIN, N]
        arep = np.zeros((128, NT), f32)
        for p in range(128):
            for k in range(NT):
                arep[p, k] = A[8 * k + p // 16, p % 16]
        blob[f"Arep_{w}"] = arep

    bblob = {}
    outpT = np.ascontiguousarray(np.asarray(inputs["out_proj_w"]).T).astype(f32)
    a = np.zeros((128, C), f32)
    b = np.zeros((128, C), f32)
    a[:, :] = outpT[0:128]
    b[64:128, :] = outpT[128:192]
    bblob["outpT_a"], bblob["outpT_b"] = a, b
    fw = np.zeros((128, C), f32)
    fw[0:C, :] = np.ascontiguousarray(np.asarray(inputs["fus_w"]).T).astype(f32)
    bblob["fuswT"] = fw
    for t in range(16):
        bblob[f"red128b_{t}"] = red[t]
    for v in range(8):
        bblob[f"ohsb{v}"] = blobr[f"ohs{v}"]

    shared = {
        "mean96": np.full((C, C), 1.0 / C, f32),
        "w1T": np.ascontiguousarray(W1big.T).astype(f32),
    }

    in_maps = []
    for s in range(NS):
        m = dict(shared)
        xs = np.zeros((C, TP), f32)
        lo = s * SL - 3
        if lo < 0:
            xs[:, 3:] = x[:, 0:SL]
        else:
            xs[:, :] = x[:, lo:(s + 1) * SL]
        m["x_sl"] = xs
        pf = np.zeros((DIN, 3), f32)
        if s == 0:
            pf[:, :] = np.float32(-bW[:DIN, None])
        bl = dict(blob)
        bl["padfix_a"], bl["padfix_b"] = split_ab(pf)
        for w in ("f", "r"):
            j = s - 1 if w == "f" else s + 1
            sel = np.zeros((128, 1), f32)
            if 0 <= j < NS:
                sel[j, 0] = 1.0
            bl[f"sel_{w}"] = sel
        bf = np.zeros((128, F32_COLS), f32)
        for nm, (o, ncol) in _F32_OFF.items():
            bf[:, o:o + ncol] = bl[nm]
        m["blobf"] = bf
        br = np.zeros((128, F32R_COLS), f32)
        for nm, (o, ncol) in _F32R_OFF.items():
            br[:, o:o + ncol] = blobr[nm]
        m["blobr"] = br
        bb = np.zeros((128, BF_COLS), f32)
        for nm, (o, ncol) in _BF_OFF.items():
            bb[:, o:o + ncol] = bblob[nm]
        m["blobb"] = bb.astype(ml_dtypes.bfloat16)
        in_maps.append(m)
    return in_maps


def run_cores(inputs, dbg=False, trace=False):
    from concourse.bass_utils import run_bass_kernel_spmd
    key = ("g", dbg)
    if key not in _cache:
        _cache[key] = _build_graph(dbg=dbg)
    nc, dbg_t = _cache[key]
    in_maps = _host_prep(inputs)
    res = run_bass_kernel_spmd(nc, in_maps, core_ids=list(range(NS)), trace=trace)
    return res, dbg_t


def kernel(**inputs):
    res, _ = run_cores(inputs, dbg=False, trace=False)
    out = np.zeros((C, NS * SL), np.float32)
    for s in range(NS):
        out[:, s * SL:(s + 1) * SL] = res.results[s]["out"]
    return out.reshape(1, C, 8, 32, 32)
